# revision 40
# baseline (speedup 1.0000x reference)
"""Trainium2 Bass kernel: backward of mHC post-mixing
    y = h_res @ x + h_post[..., :, None] * h_out[..., None, :]

Four fused gradients, fp32 accumulation, bf16 outputs.

Shapes (full): B=4, S=2048, N=4, D=2048.  Tokens T = B*S = 8192.
Sharding: pure data parallel over tokens, 8 cores x 1024 tokens.

Per-core compute layout (tokens processed in 8 supergroups of 128 tokens,
each supergroup = 4 groups of 32 tokens; partition dim = 32 tokens x 4):

 Phase A (token-major, rows = t*4+i):
   grad_x[t,j,:]   = sum_i h_res[t,i,j] g[t,i,:]   -> PE matmul with a
        host-packed [128,128] block-diagonal of h_res as stationary lhsT.
   grad_h_out[t,:] = sum_i h_post[t,i] g[t,i,:]    -> PE matmul with a
        host-packed block-diagonal of h_post (zero-padded to 64 columns
        per group because PE outputs can only start at partition 0/32/64).
 Phase B (d-major via DMA-transpose loads, contraction over D in 16
   chunks of 128 on the PE partition dim):
   grad_h_res[t,i,j] = <g[t,i,:], x[t,j,:]>   -> [128,128] matmul per
        (group, chunk) accumulated in PSUM; block-diagonal extracted with
        a mask multiply + grouped reduction on DVE.
   grad_h_post[t,i]  = <g[t,i,:], h_out[t,:]> -> [128,32] matmul, same
        extraction trick.

DMA orchestration: the XPOSE (DmaTransposeAnt) ISA encoding fits only ONE
sync-wait command, and Tile's semaphore assignment elides waits only
against ticks already observed on the same DMA-ring stream.  So all
regular copies ride the software DGE (gpsimd) using the DMASW lanes,
while the sync-engine HWDGE ring carries EXACTLY eight DMAs per
supergroup -- five tiny "ring gate" copies plus the three transposes,
strictly ordered -- so the DMAHW lane rotation is periodic and each
gate's two dependencies (same-position instructions of the previous
supergroup) keep every lane's latest tick observed.  The transposes then
emit at most one wait (the copy->xpose transition).
"""

import numpy as np
import ml_dtypes

import concourse.bass as bass
import concourse.mybir as mybir
from concourse.tile import TileContext
from concourse.tile_rust import add_dep_helper
from concourse.bass_utils import run_bass_kernel_spmd

BF16 = ml_dtypes.bfloat16

# Problem constants (hardcoded; kernel.py must be self-contained).
B1, S, N, D = 4, 2048, 4, 2048
T = B1 * S                  # 8192 tokens
NCORES = 8
TC = T // NCORES            # 1024 tokens per core
NSG = TC // 128             # 8 supergroups of 128 tokens
P = 128
NCHUNK = D // P             # 16 d-chunks

_DT = mybir.dt.bfloat16
_F32 = mybir.dt.float32


def _raw(ins):
    if isinstance(ins, (list, tuple)):
        ins = ins[-1]
    if isinstance(ins, bass.BassInstruction):
        return ins.ins
    return ins


def _dep(a, b, sync=True, reason="dep"):
    if b is None or a is None:
        return
    if isinstance(a, (list, tuple)):
        for aa in a:
            _dep(aa, b, sync, reason)
        return
    if isinstance(b, (list, tuple)):
        for bb in b:
            _dep(a, bb, sync, reason)
        return
    add_dep_helper(_raw(a), _raw(b), sync, reason=reason)


def _pe_gate(nc, dummy_w, deps, prev=None, hint="pe_gate"):
    """PE-stream wait absorber built from tiny standalone ldweights.

    The Matmult ISA encoding fits only one sync-wait command, but PSUM
    slot recycling gives a matmul two cross-proc dependencies (the old
    matmul's PE tick and its PSUM-reader's tick).  NoOps are not
    tick-assigned, so they cannot advance the PE stream's observed
    vector clock -- only a real PE instruction can.  A chain of dummy
    [1,1] ldweights, one dependency each, pre-waits those ticks; the
    following real matmul then emits at most one wait.  (Scheduling
    keeps the chain between whole matmuls; the legalizer splits
    ldweights+matmult pairs only afterwards, so the dummy never lands
    inside a real pair.)
    """
    last = prev
    for d in deps:
        if d is None:
            continue
        lw = nc.tensor.ldweights(dummy_w[0:1, 0:1])
        _dep(lw, d, reason=hint)
        if last is not None:
            _dep(lw, last, False, reason=hint + " order")
        last = lw
    return last


def _build_nc() -> bass.Bass:
    nc = bass.Bass("TRN2")

    g = nc.dram_tensor("g", [TC * N, D], _DT, kind="ExternalInput")
    xin = nc.dram_tensor("xin", [TC * N, D], _DT, kind="ExternalInput")
    ho = nc.dram_tensor("ho", [TC, D], _DT, kind="ExternalInput")
    # per-supergroup packed stationary weights: [bd (512) | hp (256)]
    bdhp = nc.dram_tensor("bdhp", [NSG, P, 768], _DT, kind="ExternalInput")
    mres = nc.dram_tensor("mres", [P, 512], _DT, kind="ExternalInput")
    mpost = nc.dram_tensor("mpost", [P, P], _DT, kind="ExternalInput")

    gx = nc.dram_tensor("gx", [TC * N, D], _DT, kind="ExternalOutput")
    gho = nc.dram_tensor("gho", [TC, D], _DT, kind="ExternalOutput")
    grh = nc.dram_tensor("grh", [P, NSG * 16], _DT, kind="ExternalOutput")
    ghp = nc.dram_tensor("ghp", [P, NSG * 4], _DT, kind="ExternalOutput")

    mult = mybir.AluOpType.mult
    add = mybir.AluOpType.add

    with TileContext(nc) as tc:
        with tc.tile_pool(name="const", bufs=1) as cpool, \
             tc.tile_pool(name="gin", bufs=2) as gpool, \
             tc.tile_pool(name="tin", bufs=2) as tpool, \
             tc.tile_pool(name="outp", bufs=2) as opool, \
             tc.tile_pool(name="psA", bufs=2, space="PSUM") as psA, \
             tc.tile_pool(name="psB", bufs=2, space="PSUM") as psB:

            mres_t = cpool.tile([P, 512], _DT)
            nc.gpsimd.dma_start(mres_t, mres[:])
            mpost_t = cpool.tile([P, P], _DT)
            nc.gpsimd.dma_start(mpost_t, mpost[:])
            grh_sb = cpool.tile([P, NSG * 16], _DT)
            ghp_sb = cpool.tile([P, NSG * 4], _DT)
            rg_sb = cpool.tile([1, 40], _DT)
            dummy_w = cpool.tile([1, 8], _DT)
            nc.gpsimd.dma_start(dummy_w, mres[0:1, 0:8])
            act_scr = cpool.tile([1, 8], _DT)
            sw_scr = cpool.tile([1, 32], _DT)
            dve_scr = cpool.tile([1, 8], _DT)
            # warmup gates: absorb the scratch/mask load ticks into each
            # engine stream so first users emit at most one wait
            nc.tensor.ldweights(dummy_w[0:1, 0:1])
            dve_g1 = nc.vector.tensor_copy(dve_scr[0:1, 0:1], mres_t[0:1, 0:1])
            dve_g2 = nc.vector.tensor_copy(dve_scr[0:1, 1:2], mpost_t[0:1, 0:1])
            _dep(dve_g2, dve_g1, False, reason="dve warm order")
            nc.scalar.copy(act_scr[0:1, 0:1], dummy_w[0:1, 0:1])

            hw_recent = []           # last 8 HWDGE-ring DMAs (all rings)
            last_ring_inst = None
            phaseb_mms = {}
            extract_dve = {}
            gx_copies = []
            gho_copies = []
            sb_releases = {}    # sg -> insts releasing gx_sb/gho_sb slots
            act_gate_last = {}
            swdge_recent = []   # last 8 SWDGE DMAs (covers all DMASW lanes)
            phasea_mms = {}

            for sg in range(NSG):
                r0 = sg * 512   # first row of this supergroup in [TC*N, D]

                # ---- HWDGE sync ring: gates + 3 batched transposes ----
                # Each gate carries at most ONE explicit dependency (the
                # DMACopy encoding fits two sync waits and a gate may also
                # need its own-lane recycle wait).  Depending on the last
                # eight hardware-ring DMAs keeps every DMAHW lane's latest
                # tick observed on this ring before the transposes issue;
                # one gate absorbs the PE slot-recycle tick.
                gate_deps = [None] + hw_recent[-8:] + ["PB"]
                ring = []
                for k, gd in enumerate(gate_deps):
                    col = 8 * (k % 5)
                    rg = nc.sync.dma_start(rg_sb[0:1, col:col + 8],
                                           mres[0:1, col:col + 8])
                    if gd == "PB":
                        for mm in phaseb_mms.get(sg - 2, []):
                            _dep(rg, mm, reason="slot recycle PE dep")
                    elif gd is not None:
                        _dep(rg, gd, reason="ring gate lane dep")
                    _dep(rg, ring[-1] if ring else last_ring_inst, False,
                         reason="ring order")
                    ring.append(rg)

                gt_a = tpool.tile([P, NCHUNK * 512], _DT, tag="gt", bufs=2)
                xt_a = tpool.tile([P, NCHUNK * 512], _DT, tag="xt", bufs=2)
                ht_a = tpool.tile([P, NCHUNK * P], _DT, tag="ht", bufs=2)
                # out[p, c, r] = in[r, 128c + p]: chunk c of the transposed
                # slab lands at cols [512c, 512c+512).
                xp1 = nc.sync.dma_start_transpose(
                    gt_a.rearrange("p (c r) -> p c r", c=NCHUNK),
                    g[r0:r0 + 512, :])
                xp2 = nc.sync.dma_start_transpose(
                    xt_a.rearrange("p (c r) -> p c r", c=NCHUNK),
                    xin[r0:r0 + 512, :])
                xp3 = nc.sync.dma_start_transpose(
                    ht_a.rearrange("p (c r) -> p c r", c=NCHUNK),
                    ho[sg * P:(sg + 1) * P, :])
                for xp in (xp1, xp2, xp3):
                    _dep(xp, ring[-1], False, reason="ring order")
                    ring.append(xp)
                hw_recent = (hw_recent + ring)[-8:]
                last_ring_inst = ring[-1]

                gt_c = [gt_a[:, 512 * c:512 * (c + 1)] for c in range(NCHUNK)]
                xt_c = [xt_a[:, 512 * c:512 * (c + 1)] for c in range(NCHUNK)]
                ht_c = [ht_a[:, P * c:P * (c + 1)] for c in range(NCHUNK)]

                # ---- SWDGE loads: token-major g slab, packed weights ----
                # A few tiny gate copies keep the DMASW lane ticks and the
                # PE slot-recycle tick observed so the big loads emit at
                # most two waits.
                sw_prev = xp3
                sw_deps = swdge_recent[-6:] + ["PA"]
                for ci, gd in enumerate(sw_deps):
                    sgate = nc.gpsimd.dma_start(
                        sw_scr[0:1, 8 * (ci % 4):8 * (ci % 4) + 8],
                        mres[0:1, 8 * (ci % 4):8 * (ci % 4) + 8])
                    if gd == "PA":
                        for mm in phasea_mms.get(sg - 2, []):
                            _dep(sgate, mm, reason="g_sg slot PE dep")
                    else:
                        _dep(sgate, gd, reason="swdge gate dep")
                    _dep(sgate, sw_prev, False, reason="swdge gate order")
                    sw_prev = sgate
                g_sg = gpool.tile([P, N * D], _DT, tag="g_sg", bufs=2)
                ldg = nc.gpsimd.dma_start(
                    g_sg.rearrange("p (a d) -> p a d", a=4),
                    g[r0:r0 + 512, :].rearrange("(a p) d -> p a d", p=P))
                _dep(ldg, sw_prev, False, reason="after swdge gates")
                g_t = [g_sg[:, D * k2:D * (k2 + 1)] for k2 in range(4)]
                bdhp_t = gpool.tile([P, 768], _DT, tag="bdhp", bufs=2)
                ldw = nc.gpsimd.dma_start(bdhp_t, bdhp[sg])
                _dep(ldw, sw_prev, False, reason="after swdge gates")
                swdge_recent = (swdge_recent + [ldg, ldw])[-6:]
                bd_t = bdhp_t[:, 0:512]
                hp_t = bdhp_t[:, 512:768]

                # ---- Phase A: grad_x ----
                # Each PSUM slot recycle makes the next matmul depend on both
                # the old matmul (PE) and its ACT copy; the matmul encoding
                # fits one wait, so a PE NoOp pre-waits the ACT tick.
                gx_sb = opool.tile([P, N * D], _DT, tag="gx_sb", bufs=2)
                pa_gate = _pe_gate(nc, dummy_w, [ldg, ldw],
                                   hint=f"pa_in_gate_{sg}")
                # ACT-stream absorber for the gx_sb/gho_sb slot releases
                act_prev = None
                if act_gate_last.get(sg - 1) is not None:
                    ag0 = nc.scalar.copy(act_scr[0:1, 7:8],
                                         dummy_w[0:1, 0:1])
                    _dep(ag0, act_gate_last[sg - 1],
                         reason="act stream catchup")
                    act_prev = ag0
                for kk, d in enumerate(sb_releases.get(sg - 2, [])):
                    ag = nc.scalar.copy(act_scr[0:1, kk:kk + 1],
                                        dummy_w[0:1, 0:1])
                    _dep(ag, d, reason="sb slot release")
                    _dep(ag, act_prev, False, reason="act gate order")
                    act_prev = ag
                act_gate_last[sg] = act_prev
                first_copy = [True]
                pa_list = []
                for k2 in range(4):
                    for c in range(4):
                        n = 4 * k2 + c
                        if len(gx_copies) >= 2:
                            pg = _pe_gate(nc, dummy_w, [gx_copies[-2]],
                                          hint="gx_slot_gate")
                        else:
                            pg = pa_gate
                        ps = psA.tile([P, 512], _F32, tag="ps_gx", bufs=2)
                        mm = nc.tensor.matmul(
                            ps[:],
                            lhsT=bd_t[:, P * k2:P * (k2 + 1)],
                            rhs=g_t[k2][:, 512 * c:512 * (c + 1)],
                            start=True, stop=True)
                        _dep(mm, pg, False, reason="after slot gate")
                        pa_list.append(mm)
                        cp = nc.scalar.copy(
                            gx_sb[:, D * k2 + 512 * c: D * k2 + 512 * (c + 1)],
                            ps[:])
                        if first_copy[0] and act_prev is not None:
                            _dep(cp, act_prev, False, reason="after act gate")
                            first_copy[0] = False
                        gx_copies.append(cp)
                ag_st = nc.scalar.dma_start(rg_sb[0:1, 32:40],
                                            mres[0:1, 32:40])
                st_gx = nc.scalar.dma_start(
                    gx[r0:r0 + 512, :].rearrange("(a p) d -> p a d", p=P),
                    gx_sb.rearrange("p (a d) -> p a d", a=4))
                _dep(st_gx, ag_st, False, reason="after store gate")
                hw_recent = (hw_recent + [ag_st, st_gx])[-8:]

                # ---- Phase A: grad_h_out ----
                # PE outputs can only start at partition 0/32/64, so each
                # group's lhsT is zero-padded to 64 columns; group pairs land
                # at bases 0 and 64 and accumulate into place (adding zeros
                # over the other group's tokens).
                gho_sb = opool.tile([P, D], _DT, tag="gho_sb", bufs=2)
                for c in range(4):
                    if len(gho_copies) >= 2:
                        pg = _pe_gate(nc, dummy_w, [gho_copies[-2]],
                                      hint="gho_slot_gate")
                    else:
                        pg = pa_gate
                    ps = psA.tile([P, 512], _F32, tag="ps_gho", bufs=2)
                    for k2 in range(4):
                        mm = nc.tensor.matmul(
                            ps[64 * (k2 // 2):64 * (k2 // 2) + 64, :],
                            lhsT=hp_t[:, 64 * k2:64 * (k2 + 1)],
                            rhs=g_t[k2][:, 512 * c:512 * (c + 1)],
                            start=(k2 % 2 == 0), stop=(k2 % 2 == 1),
                            skip_group_check=True)
                        if k2 == 0:
                            _dep(mm, pg, False, reason="after slot gate")
                        pa_list.append(mm)
                    cpg = nc.scalar.copy(
                        gho_sb[:, 512 * c:512 * (c + 1)], ps[:])
                    if c == 0 and act_prev is not None:
                        _dep(cpg, act_prev, False, reason="after act gate")
                    gho_copies.append(cpg)
                phasea_mms[sg] = pa_list
                st_gho = nc.scalar.dma_start(gho[sg * P:(sg + 1) * P, :],
                                             gho_sb[:])
                _dep(st_gho, st_gx, False, reason="store order")
                hw_recent = (hw_recent + [st_gho])[-8:]
                sb_releases[sg] = [gx_copies[-1], st_gx, gho_copies[-1],
                                   st_gho]

                # ---- Phase B: grad_h_res / grad_h_post ----
                # k2-outer, chunk-inner so each group's PSUM accumulation
                # group is complete before the next start=True clears the
                # bank's has_written bits.
                pb_gate = _pe_gate(
                    nc, dummy_w,
                    [xp1, xp2, xp3] + extract_dve.get(sg - 2, []),
                    hint=f"pb_gate_{sg}")
                ps_res = psB.tile([P, 512], _F32, tag="ps_res", bufs=2)
                ps_post = psB.tile([P, P], _F32, tag="ps_post", bufs=2)
                mms = []
                for k2 in range(4):
                    for c in range(NCHUNK):
                        lw = gt_c[c][:, P * k2:P * (k2 + 1)]
                        if not mms:
                            pass
                        mms.append(nc.tensor.matmul(
                            ps_res[:, P * k2:P * (k2 + 1)],
                            lhsT=lw,
                            rhs=xt_c[c][:, P * k2:P * (k2 + 1)],
                            start=(c == 0), stop=(c == NCHUNK - 1),
                            skip_group_check=True))
                        mms.append(nc.tensor.matmul(
                            ps_post[:, 32 * k2:32 * (k2 + 1)],
                            lhsT=lw,
                            rhs=ht_c[c][:, 32 * k2:32 * (k2 + 1)],
                            start=(c == 0), stop=(c == NCHUNK - 1),
                            skip_group_check=True))
                _dep(mms[0], pb_gate, False, reason="after pb gate")
                _dep(mms[1], pb_gate, False, reason="after pb gate")
                phaseb_mms[sg] = mms

                # extraction: mask-multiply then grouped reduce over t'
                tmp_res = opool.tile([P, 512], _DT, tag="tmp_res", bufs=2)
                tt1 = nc.vector.tensor_tensor(tmp_res[:], ps_res[:], mres_t[:],
                                              mult)
                red_res = opool.tile([P, 16], _F32, tag="red_res", bufs=2)
                nc.vector.tensor_reduce(
                    red_res[:],
                    tmp_res.rearrange("p (a b c) -> p a c b", a=4, b=32),
                    axis=mybir.AxisListType.X, op=add)
                nc.vector.tensor_copy(grh_sb[:, 16 * sg:16 * (sg + 1)],
                                      red_res[:])

                tmp_post = opool.tile([P, P], _DT, tag="tmp_post", bufs=2)
                tt2 = nc.vector.tensor_tensor(tmp_post[:], ps_post[:],
                                              mpost_t[:], mult)
                red_post = opool.tile([P, 4], _F32, tag="red_post", bufs=2)
                nc.vector.tensor_reduce(
                    red_post[:],
                    tmp_post.rearrange("p (a b) -> p a b", a=4),
                    axis=mybir.AxisListType.X, op=add)
                nc.vector.tensor_copy(ghp_sb[:, 4 * sg:4 * (sg + 1)],
                                      red_post[:])
                extract_dve[sg] = [tt1, tt2]

            nc.scalar.dma_start(grh[:], grh_sb[:])
            nc.scalar.dma_start(ghp[:], ghp_sb[:])

    _split_excess_waits(nc)
    return nc


_WAIT_LIMITS = {
    "InstDMACopy": 1,
    "InstDmaTransposeAnt": 1,
    "InstMatmult": 1,
    "InstLdweights": 1,
    "InstActivation": 1,
    "InstTensorTensor": 1,
    "InstTensorCopy": 1,
    "InstTensorReduce": 1,
    "InstTensorScalarPtr": 1,
    "InstTensorTensorReduce": 1,
    "InstMemSet": 1,
    "InstDrain": 1,
    "InstEventSemaphore": 1,
}


def _split_excess_waits(nc):
    """Post-schedule legalization: move excess sync-waits onto injected
    NoOps.

    Each TPB instruction encoding fits a limited number of sync-wait
    commands (one for compute/XPOSE, two for DMACopy); walrus codegen
    hard-fails beyond that.  A NoOp on the same engine right before the
    instruction executes the same waits in the same stream order, so
    splitting is runtime-equivalent and keeps walrus happy.
    """
    for fn in nc.m.functions:
        for blk in fn.blocks:
            insts = blk.instructions
            out = []
            changed = False
            for ins in insts:
                si = ins.sync_info
                lim = _WAIT_LIMITS.get(type(ins).__name__)
                if si is not None and lim is not None and \
                        len(si.on_wait) > lim:
                    waits = list(si.on_wait)
                    excess = waits[:-lim] if lim else waits
                    kept = waits[-lim:] if lim else []
                    # one wait per NoOp: the 64-byte TPB encoding has a
                    # single sync-wait slot
                    for wexc in excess:
                        noop = mybir.InstNoOp(
                            name=nc.get_next_instruction_name(),
                            text_hint="wait_split", bass_nofuse=True)
                        noop.engine = ins.engine
                        noop.sync_info = type(si)(
                            on_wait=[wexc], on_update=[])
                        out.append(noop)
                    ins.sync_info = type(si)(
                        on_wait=kept, on_update=list(si.on_update))
                    changed = True
                out.append(ins)
            if changed:
                blk.instructions = out


_NC = None


def _get_nc():
    global _NC
    if _NC is None:
        _NC = _build_nc()
    return _NC


def _host_pack(h_res_c, h_post_c):
    """Pack per-core h_res/h_post into block-diagonal stationary weights."""
    bd = np.zeros((NSG, P, 512), dtype=BF16)
    hp = np.zeros((NSG, P, 256), dtype=BF16)
    sg_, k2_, t_, i_, j_ = np.ix_(
        np.arange(NSG), np.arange(4), np.arange(32), np.arange(4),
        np.arange(4))
    tok = 128 * sg_ + 32 * k2_ + t_
    bd[sg_, 4 * t_ + i_, 128 * k2_ + 4 * t_ + j_] = h_res_c[tok, i_, j_]
    sg_, k2_, t_, i_ = np.ix_(
        np.arange(NSG), np.arange(4), np.arange(32), np.arange(4))
    tok = 128 * sg_ + 32 * k2_ + t_
    # lhsT slice for group k2 spans cols [64*k2, 64*k2+64); within it the
    # column for local token t is 32*(k2 % 2) + t (out bases 0/0/64/64).
    hp[sg_, 4 * t_ + i_, 64 * k2_ + 32 * (k2_ % 2) + t_] = h_post_c[tok, i_]
    return np.ascontiguousarray(np.concatenate([bd, hp], axis=2))


def _masks():
    pp = np.arange(P)
    tprime = np.arange(32)
    m = (tprime[None, :] == (pp[:, None] // 4)).astype(BF16)   # [128, 32]
    mres = np.tile(np.repeat(m, 4, axis=1), (1, 4))            # [128, 512]
    mpost = np.tile(m, (1, 4))                                 # [128, 128]
    return np.ascontiguousarray(mres), np.ascontiguousarray(mpost)


def kernel(grad_output, x, h_res, h_out, h_post):
    g_f = np.asarray(grad_output).reshape(T, N, D)
    x_f = np.asarray(x).reshape(T, N, D)
    hr_f = np.asarray(h_res).reshape(T, N, N)
    hout_f = np.asarray(h_out).reshape(T, D)
    hp_f = np.asarray(h_post).reshape(T, N)

    mres, mpost = _masks()

    in_maps = []
    for c in range(NCORES):
        sl = slice(TC * c, TC * (c + 1))
        in_maps.append({
            "g": np.ascontiguousarray(g_f[sl].reshape(TC * N, D)),
            "xin": np.ascontiguousarray(x_f[sl].reshape(TC * N, D)),
            "ho": np.ascontiguousarray(hout_f[sl]),
            "bdhp": _host_pack(hr_f[sl], hp_f[sl]),
            "mres": mres,
            "mpost": mpost,
        })

    nc = _get_nc()
    res = run_bass_kernel_spmd(nc, in_maps, core_ids=list(range(NCORES)))
    outs = res.results

    grad_x = np.empty((T, N, D), dtype=BF16)
    grad_h_out = np.empty((T, D), dtype=BF16)
    grad_h_res = np.empty((T, N, N), dtype=BF16)
    grad_h_post = np.empty((T, N), dtype=BF16)

    for c in range(NCORES):
        sl = slice(TC * c, TC * (c + 1))
        grad_x[sl] = outs[c]["gx"].reshape(TC, N, D)
        grad_h_out[sl] = outs[c]["gho"]
        # grh staging: [p = 4t+i, 16 sg + 4 k2 + j]
        v = outs[c]["grh"].reshape(32, 4, NSG, 4, 4)     # [t, i, sg, k2, j]
        grad_h_res[sl] = v.transpose(2, 3, 0, 1, 4).reshape(TC, N, N)
        v2 = outs[c]["ghp"].reshape(32, 4, NSG, 4)       # [t, i, sg, k2]
        grad_h_post[sl] = v2.transpose(2, 3, 0, 1).reshape(TC, N)

    return (grad_x.reshape(B1, S, N, D),
            grad_h_res.reshape(B1, S, N, N),
            grad_h_out.reshape(B1, S, D),
            grad_h_post.reshape(B1, S, N))
